# revision 1
# baseline (speedup 1.0000x reference)
"""FWHT kernel for Trainium2: y = FWHT(x) along last axis.

x: (8192, 4096) fp32. Sharded row-wise (data-parallel) across 8 NeuronCores.

Math: FWHT (natural/Hadamard order) along an axis of 4096 equals
multiplication by Sylvester H_4096 = H_128 (x) H_32 (Kronecker).
Per row r (viewing x[r] as a 128x32 matrix X with j = 32*j1 + j0):
  Y = H_128 @ X @ H_32,   y[r, 32*i1 + i0] = Y[i1, i0]

Raw-bass 4-engine pipeline, per 16-row tile (free dim = 16*32 = 512):
  SYNC   : DMA load  x[16 rows] as [j1=128 part, (r,j0)=512 free] (128B runs)
  TENSOR : MM-B  Z = H128^T @ X  (contract j1; H sym)  -> PSUM zz[i%2]
  VECTOR : 32x32 block transpose Z -> T (j0 to partitions) -> SBUF tt[i%4]
  TENSOR : MM-A  W = blockdiag(H32 x4)^T @ T (contract j0) -> PSUM ww[i%2]
  VECTOR : 32x32 block transpose W -> O (i1 to partitions) -> SBUF oo[i%4]
  SCALAR : DMA store O as y[16 rows] [i1=128 part, (r,i0)=512 free]

Semaphores: load_sem/store_sem (+16 per DMA), pe_sem/dve_sem (+1 per op).
pe_sem after iter i: MM-B=2i+1, MM-A=2i+2. dve_sem: t=2i+1, o=2i+2.
"""

import numpy as np

N_CORES = 8
ROWS = 8192
COLS = 4096
ROWS_PER_CORE = ROWS // N_CORES  # 1024
R_TILE = 16                      # rows per matmul tile -> free dim 512
N_ITERS = ROWS_PER_CORE // R_TILE

B_IN = 8    # xin slots (prefetch depth)
B_MID = 4   # tt slots
B_OUT = 4   # oo slots

# "f32" native (4 cyc/row, exact fp32) or "f32r" (1 cyc/row at N>=256,
# ~1.5e-4 rel err; inputs pre-rounded on GPSIMD). The kernel is DMA-bound
# (strided 128B-run access pattern), so both run at the same speed; f32
# is exact.
MM_DTYPE = "f32"


def _sylvester(n: int) -> np.ndarray:
    H = np.array([[1.0]], dtype=np.float32)
    while H.shape[0] < n:
        H = np.block([[H, H], [H, -H]])
    return H.astype(np.float32)


def _h_weights() -> np.ndarray:
    h1 = _sylvester(128)
    h2 = np.kron(np.eye(4, dtype=np.float32), _sylvester(32)).astype(np.float32)
    return np.ascontiguousarray(np.concatenate([h1, h2], axis=1))


def _build_nc(n_iters: int = N_ITERS):
    import concourse.bass as bass
    import concourse.mybir as mybir

    f32 = mybir.dt.float32
    f32r = mybir.dt.float32r
    mm_dt = f32r if MM_DTYPE == "f32r" else f32

    # detect_race_conditions=False: the sim's sem-race check requires the
    # issuing engine to re-observe a semaphore between increments.  Our waits
    # use sum semantics (each DMA adds exactly +16, split 1 per SDMA engine,
    # per-engine FIFO), so >= 16*k implies the first k DMAs completed.
    nc = bass.Bass(detect_race_conditions=False)
    rows_total = n_iters * R_TILE
    x = nc.declare_dram_parameter("x", [rows_total, COLS], f32, isOutput=False)
    # h[:, 0:128] = H128, h[:, 128:256] = blockdiag(H32 x 4)
    h = nc.declare_dram_parameter("h", [128, 256], f32, isOutput=False)
    y = nc.declare_dram_parameter("y", [rows_total, COLS], f32, isOutput=True)

    use_f32r = MM_DTYPE == "f32r"
    with (
        nc.sbuf_tensor("ht", [128, 256], f32) as ht,
        nc.sbuf_tensor("htr", [128, 256], mm_dt) as htr,
        nc.sbuf_tensor("xin", [128, B_IN * 512], f32) as xin,
        nc.sbuf_tensor("xr", [128, B_IN * 512], mm_dt) as xr,
        nc.sbuf_tensor("tt", [128, B_MID * 512], f32) as tt,
        nc.sbuf_tensor("tr", [128, B_MID * 512], mm_dt) as tr,
        nc.sbuf_tensor("oo", [128, B_OUT * 512], f32) as oo,
        nc.psum_tensor("zz", [128, 2 * 512], f32) as zz,
        nc.psum_tensor("ww", [128, 2 * 512], f32) as ww,
        nc.semaphore("load_sem") as load_sem,
        nc.semaphore("store_sem") as store_sem,
        nc.semaphore("pe_sem") as pe_sem,
        nc.semaphore("dve_sem") as dve_sem,
        nc.semaphore("pool_sem") as pool_sem,
        nc.Block() as block,
    ):
        def slot(buf, i, n):
            return buf[:, (i % n) * 512:(i % n + 1) * 512]

        def xslot(i):
            return slot(xin, i, B_IN)

        def tslot(i):
            return slot(tt, i, B_MID)

        def oslot(i):
            return slot(oo, i, B_OUT)

        def zslot(i):
            return slot(zz, i, 2)

        def wslot(i):
            return slot(ww, i, 2)

        @block.sync
        def _(sync):
            sync.dma_start(ht[:], h[:]).then_inc(load_sem, 16)
            for i in range(n_iters):
                if i >= B_IN:
                    # xin slot reader must be done: MM-B(i-B_IN) (f32) or
                    # GPSIMD rounding copy (f32r)
                    if use_f32r:
                        sync.wait_ge(pool_sem, 2 * (i - B_IN) + 2)
                    else:
                        sync.wait_ge(pe_sem, 2 * (i - B_IN) + 1)
                rows = x[i * R_TILE:(i + 1) * R_TILE, :]
                sync.dma_start(
                    xslot(i).rearrange("p (r j0) -> p r j0", j0=32),
                    rows.rearrange("r (j1 j0) -> j1 r j0", j0=32),
                ).then_inc(load_sem, 16)

        if use_f32r:
            @block.gpsimd
            def _(gpsimd):
                # round weights once: pool_sem -> 1
                gpsimd.wait_ge(load_sem, 16)
                gpsimd.tensor_copy(htr[:], ht[:]).then_inc(pool_sem)
                for i in range(n_iters):
                    # round xin(i) -> xr(i): pool_sem -> 2i+2
                    gpsimd.wait_ge(load_sem, 16 * (i + 2))
                    if i >= B_IN:
                        # xr slot reader (MM-B of iter i-B_IN) must be done
                        gpsimd.wait_ge(pe_sem, 2 * (i - B_IN) + 1)
                    gpsimd.tensor_copy(
                        slot(xr, i, B_IN), xslot(i)
                    ).then_inc(pool_sem)
                    # round t(i) -> tr(i): pool_sem -> 2i+3
                    gpsimd.wait_ge(dve_sem, 2 * i + 1)
                    gpsimd.tensor_copy(
                        slot(tr, i, B_MID), tslot(i)
                    ).then_inc(pool_sem)

        @block.scalar
        def _(scalar):
            for i in range(n_iters):
                scalar.wait_ge(dve_sem, 2 * i + 2)  # o(i) ready
                yrows = y[i * R_TILE:(i + 1) * R_TILE, :]
                scalar.dma_start(
                    yrows.rearrange("r (i1 i0) -> i1 r i0", i0=32),
                    oslot(i).rearrange("p (r i0) -> p r i0", i0=32),
                ).then_inc(store_sem, 16)

        @block.tensor
        def _(tensor):
            for i in range(n_iters):
                if use_f32r:
                    tensor.wait_ge(pool_sem, 2 * i + 2)  # htr + xr(i)
                    rhs_b = slot(xr, i, B_IN)
                    lhs_b = htr[:, 0:128]
                else:
                    tensor.wait_ge(load_sem, 16 * (i + 2))  # h + xin(0..i)
                    rhs_b = xslot(i).bitcast(mm_dt)
                    lhs_b = ht[:, 0:128].bitcast(mm_dt)
                tensor.matmul(
                    out=zslot(i), lhsT=lhs_b, rhs=rhs_b, start=True, stop=True
                ).then_inc(pe_sem)  # -> 2i+1
                if use_f32r:
                    tensor.wait_ge(pool_sem, 2 * i + 3)  # tr(i) ready
                    rhs_a = slot(tr, i, B_MID)
                    lhs_a = htr[:, 128:256]
                else:
                    tensor.wait_ge(dve_sem, 2 * i + 1)  # t(i) ready
                    rhs_a = tslot(i).bitcast(mm_dt)
                    lhs_a = ht[:, 128:256].bitcast(mm_dt)
                tensor.matmul(
                    out=wslot(i), lhsT=lhs_a, rhs=rhs_a, start=True, stop=True
                ).then_inc(pe_sem)  # -> 2i+2

        @block.vector
        def _(vector):
            for i in range(n_iters):
                vector.wait_ge(pe_sem, 2 * i + 1)  # z(i) done
                vector.transpose(tslot(i), zslot(i)).then_inc(dve_sem)
                if i >= B_OUT:
                    # oo slot reader (store of iter i-B_OUT) must be done
                    vector.wait_ge(store_sem, 16 * (i - B_OUT + 1))
                vector.wait_ge(pe_sem, 2 * i + 2)  # w(i) done
                vector.transpose(oslot(i), wslot(i)).then_inc(dve_sem)

    return nc


_CACHE = {}


def kernel(x: np.ndarray) -> np.ndarray:
    from concourse.bass_utils import run_bass_kernel_spmd

    assert x.shape == (ROWS, COLS) and x.dtype == np.float32

    if "nc" not in _CACHE:
        _CACHE["nc"] = _build_nc()
    nc = _CACHE["nc"]

    h = _h_weights()

    core_ids = list(range(N_CORES))
    in_maps = [
        {
            "x": np.ascontiguousarray(x[i * ROWS_PER_CORE:(i + 1) * ROWS_PER_CORE]),
            "h": h,
        }
        for i in core_ids
    ]
    res = run_bass_kernel_spmd(nc, in_maps, core_ids)
    out = np.empty((ROWS, COLS), dtype=np.float32)
    for i in core_ids:
        out[i * ROWS_PER_CORE:(i + 1) * ROWS_PER_CORE] = res.results[i]["y"]
    return out



# revision 24
# speedup vs baseline: 2.0173x; 2.0173x over previous
"""FWHT kernel for Trainium2: y = FWHT(x) along last axis.

x: (8192, 4096) fp32. Sharded row-wise (data-parallel) across 8 NeuronCores.

Math: FWHT (natural order) is y[i] = sum_j (-1)^{<i,j>} x[j] over 12-bit
indices, which factorizes over any aligned bit split. Split j = (j1h:5 |
j1m:2 | j0:5) and i likewise:
  y[i1h,i1m,i0] = sum H32[j1h,i1h] H4[j1m,i1m] H32[j0,i0] x[j1h,j1m,j0]

Layout is chosen so every DMA descriptor is a 512B contiguous run (the DMA
cost model charges 2x below 512B). Rows are interleaved 4-way into
partition groups: partition p = rg*32 + k, with rg = row mod 4 inside a
16-row tile (row_local = 4r + rg, r in [0,4)). The HBM access pattern
[(128,128),(16384,4),(1,128)] then has 128-element (512B) contiguous runs.

Per 16-row tile (free dim 512):
  SP     : DMA load  X[p=(rg,j1h), f=(r,j1m,j0)] as f32r    (512B runs)
  TENSOR : MM1  Z = B^T X, B = I4 (x) H32, contract j1h -> PSUM zz (f32r;
           DRAM x is declared f32r so the BIR verifier accepts DMA->matmul)
  ACT    : convert zz f32 -> zb bf16 (rounding producer for MM2's inputs)
  VECTOR : T1   32x32 block transpose zb -> T[p=(rg,j0)] SBUF bf16
  TENSOR : MM2  16 matmuls: W[.,(r,i1m,i1h)] += H4[j1m,i1m]*(B^T T) (bf16)
  VECTOR : T2   block transpose -> O[p=(rg,i1h), f=(r,i1m,i0)] SBUF f32
  ACT    : DMA store O -> y                                  (512B runs)

T1/T2 are batched over tile PAIRS ([128,1024] per op) to amortize DVE
access-latency + inter-op gaps; conv and MM1 run 4 tiles ahead of the
store/MM2 stream so ACT's store SEQ waits never gate the conv chain.

Per-core DMA: 32 MB at 512B/desc = 93.2 us; per-tile engine busy: DMA
1456 ns > DVE ~1310 > PE ~1070 > ACT ~620, so the kernel is DMA-bound at
the >=512B-run roofline.

Precision: x in f32r (~1e-4 rel), Z in bf16 (~4e-3), weights are +-1
(exact); PSUM accumulation f32. Well within the 2e-2 tolerance.
"""

import numpy as np

N_CORES = 8
ROWS = 8192
COLS = 4096
ROWS_PER_CORE = ROWS // N_CORES  # 1024
R_TILE = 16                      # rows per tile -> free dim 512
N_ITERS = ROWS_PER_CORE // R_TILE  # 64

B_IN = 16   # xin slots
B_MID = 8   # zb/tt slots (pair-aligned)
B_OUT = 12  # oo slots (pair-aligned)
N_PSUM = 4  # zz/ww slots (4 banks each)
LA = 6      # MM1 lookahead (tiles) over the MM2 stream


def _sylvester(n: int) -> np.ndarray:
    H = np.array([[1.0]], dtype=np.float32)
    while H.shape[0] < n:
        H = np.block([[H, H], [H, -H]])
    return H.astype(np.float32)


def _weights():
    import ml_dtypes

    B = np.kron(np.eye(4, dtype=np.float32), _sylvester(32)).astype(np.float32)
    Bb = np.concatenate([B, -B], axis=1).astype(ml_dtypes.bfloat16)
    return np.ascontiguousarray(B), np.ascontiguousarray(Bb)


def _build_nc(n_iters: int = N_ITERS):
    import concourse.bass as bass
    import concourse.mybir as mybir

    assert n_iters % 2 == 0
    f32 = mybir.dt.float32
    f32r = mybir.dt.float32r
    bf16 = mybir.dt.bfloat16

    # detect_race_conditions=False: waits use sum semantics (each DMA adds
    # exactly +16 split across SDMA engines, per-engine FIFO), so >= 16*k
    # implies the first k DMAs completed.
    nc = bass.Bass(detect_race_conditions=False)
    rows_total = n_iters * R_TILE
    x = nc.declare_dram_parameter("x", [rows_total, COLS], f32r, isOutput=False)
    bw_d = nc.declare_dram_parameter("bw", [128, 128], f32r, isOutput=False)
    bwb_d = nc.declare_dram_parameter("bwb", [128, 256], bf16, isOutput=False)
    y = nc.declare_dram_parameter("y", [rows_total, COLS], f32, isOutput=True)

    with (
        nc.sbuf_tensor("bw_sb", [128, 128], f32r) as bw,
        nc.sbuf_tensor("bwb_sb", [128, 256], bf16) as bwb,
        nc.sbuf_tensor("xin", [128, B_IN * 512], f32r) as xin,
        nc.sbuf_tensor("tq", [128, B_MID * 512], f32) as tq,
        nc.sbuf_tensor("tt", [128, B_MID * 512], bf16) as tt,
        nc.sbuf_tensor("oo", [128, B_OUT * 512], f32) as oo,
        nc.psum_tensor("zz", [128, N_PSUM * 512], f32) as zz,
        nc.psum_tensor("ww", [128, N_PSUM * 512], f32) as ww,
        nc.semaphore("load_sem") as load_sem,
        nc.semaphore("store_sem") as store_sem,
        nc.semaphore("pool_sem") as pool_sem,
        nc.semaphore("wt_sem") as wt_sem,
        nc.semaphore("pe1_sem") as pe1_sem,
        nc.semaphore("pe2_sem") as pe2_sem,
        nc.semaphore("dve1_sem") as dve1_sem,
        nc.semaphore("dve2_sem") as dve2_sem,
        nc.Block() as block,
    ):
        def slot(buf, i, n=N_PSUM):
            return buf[:, (i % n) * 512:(i % n + 1) * 512]

        def pair(buf, k, n=N_PSUM):
            # tiles (2k, 2k+1) -> contiguous [128, 1024] view
            return buf[:, (2 * k % n) * 512:(2 * k % n + 2) * 512]

        def sub32(ap, k):
            # [128, 512] slot -> [(p), (r: stride 128, 4), (c: 1, 32)] at
            # 32-column subblock k of each 128-run
            return ap.rearrange("p (r k c) -> p r k c", k=4, c=32)[:, :, k, :]

        @block.sync
        def _(sync):
            for q in range(n_iters // 2):
                src = x[2 * q * R_TILE:(2 * q + 2) * R_TILE, :].rearrange(
                    "(r rg) (ph inner) -> (rg ph) r inner", rg=4, inner=128
                )
                ld = sync.dma_start(
                    pair(xin, q, B_IN).rearrange(
                        "p (r inner) -> p r inner", inner=128
                    ),
                    src,
                )
                if 2 * q >= B_IN:
                    ld.wait_op(pe1_sem, 2 * q - B_IN + 2, "sem-ge")
                ld.then_inc(load_sem, 16)

        @block.tensor
        def _(tensor):
            tensor.wait_ge(wt_sem, 32)  # both weight DMAs done

            def mm1(i):
                if i >= N_PSUM:
                    # slack wait (T1 runs ahead): standalone, normally already
                    # satisfied so it doesn't hold the SEQ
                    tensor.wait_ge(dve1_sem, (i - N_PSUM) // 2 + 1)
                mm = tensor.matmul(
                    out=slot(zz, i),
                    lhsT=bw[:],
                    rhs=slot(xin, i, B_IN),
                    start=True,
                    stop=True,
                )
                mm.wait_op(load_sem, 16 * (i // 2 + 1), "sem-ge")
                mm.then_inc(pe1_sem)

            def mm2(j):
                if j >= N_PSUM:
                    tensor.wait_ge(dve2_sem, (j - N_PSUM) // 2 + 1)  # slack
                for i1m in range(4):
                    for j1m in range(4):
                        neg = bin(i1m & j1m).count("1") % 2
                        lhs = bwb[:, 128:256] if neg else bwb[:, 0:128]
                        mm = tensor.matmul(
                            out=sub32(slot(ww, j), i1m),
                            lhsT=lhs,
                            rhs=sub32(slot(tt, j, B_MID), j1m),
                            start=(j1m == 0),
                            stop=(j1m == 3),
                        )
                        if i1m == 0 and j1m == 0:
                            mm.wait_op(pool_sem, j // 2 + 1, "sem-ge")
                        if i1m == 3 and j1m == 3:
                            mm.then_inc(pe2_sem)

            for i in range(min(LA, n_iters)):
                mm1(i)
            for j in range(n_iters):
                if j + LA < n_iters:
                    mm1(j + LA)
                mm2(j)

        @block.gpsimd
        def _(gpsimd):
            # weight loads via SWDGE so they never contend with the first
            # data loads for the shared HWDGE slot
            gpsimd.dma_start(bw[:], bw_d[:]).then_inc(wt_sem, 16)
            gpsimd.dma_start(bwb[:], bwb_d[:]).then_inc(wt_sem, 16)
            # f32 -> bf16 rounding copy (SBUF->SBUF, pair-batched) on the
            # otherwise-idle Pool engine; GPSIMD cannot access PSUM, so T1
            # stages Z into tq first.
            for c in range(n_iters // 2):
                if 2 * c >= B_MID:
                    gpsimd.wait_ge(pe2_sem, 2 * c - B_MID + 2)  # tt free
                cp = gpsimd.tensor_copy(pair(tt, c, B_MID), pair(tq, c, B_MID))
                cp.wait_op(dve1_sem, c + 1, "sem-ge")
                cp.then_inc(pool_sem)

        @block.scalar
        def _(scalar):
            for k in range(n_iters // 2):
                dst = y[2 * k * R_TILE:(2 * k + 2) * R_TILE, :].rearrange(
                    "(r rg) (ph inner) -> (rg ph) r inner", rg=4, inner=128
                )
                st = scalar.dma_start(
                    dst,
                    pair(oo, k, B_OUT).rearrange(
                        "p (r inner) -> p r inner", inner=128
                    ),
                )
                st.wait_op(dve2_sem, k + 1, "sem-ge")
                st.then_inc(store_sem, 16)

        @block.vector
        def _(vector):
            n_pairs = n_iters // 2

            def t1(k):
                if 2 * k >= B_MID:
                    # tq slot free: conv of B_MID//2 pairs ago done
                    vector.wait_ge(pool_sem, k - B_MID // 2 + 1)
                tr = vector.transpose(pair(tq, k, B_MID), pair(zz, k))
                tr.wait_op(pe1_sem, 2 * k + 2, "sem-ge")
                tr.then_inc(dve1_sem)

            def t2(k):
                if 2 * k >= B_OUT:
                    vector.wait_ge(store_sem, 16 * (k - B_OUT // 2 + 1))  # slack
                tr = vector.transpose(pair(oo, k, B_OUT), pair(ww, k))
                tr.wait_op(pe2_sem, 2 * k + 2, "sem-ge")
                tr.then_inc(dve2_sem)

            for k in range(min(3, n_pairs)):
                t1(k)
            for k in range(n_pairs):
                if k + 3 < n_pairs:
                    t1(k + 3)
                t2(k)


    return nc


_CACHE = {}


def kernel(x: np.ndarray) -> np.ndarray:
    from concourse.bass_utils import run_bass_kernel_spmd

    assert x.shape == (ROWS, COLS) and x.dtype == np.float32

    if "nc" not in _CACHE:
        _CACHE["nc"] = _build_nc()
    nc = _CACHE["nc"]

    bw, bwb = _weights()

    core_ids = list(range(N_CORES))
    in_maps = [
        {
            "x": np.ascontiguousarray(x[i * ROWS_PER_CORE:(i + 1) * ROWS_PER_CORE]),
            "bw": bw,
            "bwb": bwb,
        }
        for i in core_ids
    ]
    res = run_bass_kernel_spmd(nc, in_maps, core_ids)
    out = np.empty((ROWS, COLS), dtype=np.float32)
    for i in core_ids:
        out[i * ROWS_PER_CORE:(i + 1) * ROWS_PER_CORE] = res.results[i]["y"]
    return out
